# revision 20
# baseline (speedup 1.0000x reference)
"""Trainium2 Bass kernel for the bipartite GNN message-passing encoder.

Math:
  A_r = (adj == r), r = 1..5
  An_r = diag(a) A_r diag(b),  a = 1/sqrt(Nu), b = 1/sqrt(Nv)
  Hu = relu(sum_r An_r @ W_items_r^T)   [NU, M]
  Hv = relu(sum_r An_r^T @ W_users_r^T) [NI, M]
  U  = relu(Hu @ dense_W^T + relu(u_sf @ u_W1^T + u_b1) @ u_W2^T)
  V  = relu(Hv @ dense_W^T + relu(v_sf @ v_W1^T + v_b1) @ v_W2^T)

Sharding: fully collective-free 1D row split per bipartite side. Core c
owns users [500c, 500c+500) and items [500c, 500c+500) and contracts
over the FULL opposite side locally, so no partial-sum AllReduce (and no
barrier) is ever needed. Degree normalizations are folded in on the
host: b into the item-message weights, a into the user-message weights;
the outer-side factor commutes through the relu (a,b > 0) and is applied
as a per-partition scale in pass 2.

Device program = one back-to-back bf16 matmul stream (the PE is the
bottleneck at ~142us; HAM holds 2.4 GHz only while the stream is dense).
All operands arrive pre-transposed/pre-converted in consumption order.
k-tiles are processed in GROUPS laid out contiguously per partition so
one DMA loads a whole group and one DVE is_equal builds a whole group's
rating mask -- few DMAs/DVE ops means no HW-semaphore-slot convoys
(8 slots shared by all queues; many small DMAs with distant consumers
stall the just-in-time W stream). Ramp groups are small so the first
matmul issues ~3us in. Side A's pass 2 + both side-feature projections
sit at the A->B boundary; only side B's tiny pass 2 trails the stream.
"""

import sys

import numpy as np

if "/opt/trn_rl_repo" not in sys.path:
    sys.path.insert(0, "/opt/trn_rl_repo")

import concourse.bacc as bacc  # noqa: E402
import concourse.mybir as mybir  # noqa: E402
import concourse.tile as tile  # noqa: E402

FP = mybir.dt.float32
BF = mybir.dt.bfloat16

NU = NI = 4000
R = 5
M = 256
OUT = 75
SIDE = 64
FDIM = 128

NCORES = 8
B = NU // NCORES  # 500 rows per side per core
NP = 4096  # contraction dim padded to a multiple of 128 (pad rows are
#            adj=0 => every rating mask is 0 there, contributing nothing)
KT = NP // 128  # 32 contraction k-tiles (before splitting by rating)
CH = B // 4  # 125-row output chunks in pass 2
WREC = R * M  # 1280 W columns per k-tile

# k-tile groups: one adj DMA + one mask per (group, r); small ramp first
GROUPS = [(0, 2), (2, 2), (4, 4), (8, 8), (16, 8), (24, 8)]
GMAX = 8 * B  # widest group mask

AF = mybir.ActivationFunctionType
ALU = mybir.AluOpType


def build_program():
    from contextlib import ExitStack

    nc = bacc.Bacc("TRN2", target_bir_lowering=False, debug=False, num_devices=NCORES)

    # ---- I/O. adj streams: [p, k*B + u] = adj_kblock[k][p, u]; W streams:
    # [p, k*WREC + r*M + m] = W^T[r, 128k+p, m] (deg-folded). ----
    adjtu = nc.dram_tensor("adjtu", [128, KT * B], BF, kind="ExternalInput")
    adjv = nc.dram_tensor("adjv", [128, KT * B], BF, kind="ExternalInput")
    wi = nc.dram_tensor("wi", [128, KT * WREC], BF, kind="ExternalInput")
    wu = nc.dram_tensor("wu", [128, KT * WREC], BF, kind="ExternalInput")
    uft = nc.dram_tensor("uft", [FDIM, B], BF, kind="ExternalInput")
    vft = nc.dram_tensor("vft", [FDIM, B], BF, kind="ExternalInput")
    uw1t = nc.dram_tensor("uw1t", [FDIM, SIDE], BF, kind="ExternalInput")
    vw1t = nc.dram_tensor("vw1t", [FDIM, SIDE], BF, kind="ExternalInput")
    ub1 = nc.dram_tensor("ub1", [SIDE, 1], FP, kind="ExternalInput")
    vb1 = nc.dram_tensor("vb1", [SIDE, 1], FP, kind="ExternalInput")
    uw2t = nc.dram_tensor("uw2t", [SIDE, OUT], BF, kind="ExternalInput")
    vw2t = nc.dram_tensor("vw2t", [SIDE, OUT], BF, kind="ExternalInput")
    dwt = nc.dram_tensor("dwt", [M, OUT], BF, kind="ExternalInput")
    afac = nc.dram_tensor("afac", [CH, 4], FP, kind="ExternalInput")
    bfac = nc.dram_tensor("bfac", [CH, 4], FP, kind="ExternalInput")
    u_out = nc.dram_tensor("u_out", [B, OUT], FP, kind="ExternalOutput")
    v_out = nc.dram_tensor("v_out", [B, OUT], FP, kind="ExternalOutput")

    with tile.TileContext(nc) as tc, ExitStack() as ctx:
        res = ctx.enter_context(tc.tile_pool(name="res", bufs=1))
        wpool = ctx.enter_context(tc.tile_pool(name="wpool", bufs=6))
        mpool = ctx.enter_context(tc.tile_pool(name="mpool", bufs=4))
        scr = ctx.enter_context(tc.tile_pool(name="scr", bufs=3))
        # PSUM budget (8 banks): psA 2 + psB 2 + psf 2 (released before
        # pass 2 opens ps2's 4)
        psA = ctx.enter_context(tc.tile_pool(name="psA", bufs=1, space="PSUM"))
        psB = ctx.enter_context(tc.tile_pool(name="psB", bufs=1, space="PSUM"))
        psf = tc.alloc_tile_pool(name="psf", bufs=1, space="PSUM")

        # ---- adjacency: resident, group DMAs issued upfront (ACT queue) ----
        def adj_load(adj_dram, nm):
            ts = []
            for g, (k0, nk) in enumerate(GROUPS):
                at = res.tile([128, nk * B], BF, tag=f"adj{nm}{g}", name="at")
                nc.scalar.dma_start(
                    out=at[:, :], in_=adj_dram[:, k0 * B : (k0 + nk) * B]
                )
                ts.append(at)
            return ts

        adjA_t = adj_load(adjtu, "A")
        adjB_t = adj_load(adjv, "B")

        # ---- small resident loads (after adj on the ACT queue; all are
        # first needed at the A->B boundary or later) ----
        def rload(dram, p, f, dt, tag):
            t = res.tile([p, f], dt, tag=tag)
            nc.scalar.dma_start(out=t[:, :], in_=dram[:, :])
            return t

        uft_t = rload(uft, FDIM, B, BF, "uft")
        vft_t = rload(vft, FDIM, B, BF, "vft")
        uw1t_t = rload(uw1t, FDIM, SIDE, BF, "uw1t")
        vw1t_t = rload(vw1t, FDIM, SIDE, BF, "vw1t")
        ub1_t = rload(ub1, SIDE, 1, FP, "ub1")
        vb1_t = rload(vb1, SIDE, 1, FP, "vb1")
        uw2t_t = rload(uw2t, SIDE, OUT, BF, "uw2t")
        vw2t_t = rload(vw2t, SIDE, OUT, BF, "vw2t")
        afac_t = rload(afac, CH, 4, FP, "afac")
        bfac_t = rload(bfac, CH, 4, FP, "bfac")
        dwt_t = []
        for mt in range(2):
            t = res.tile([128, OUT], BF, tag=f"dwt{mt}")
            nc.scalar.dma_start(out=t[:, :], in_=dwt[mt * 128 : (mt + 1) * 128, :])
            dwt_t.append(t)

        # ---- side-feature projections (PE-tiny; emitted at the boundary) ---
        def side_proj(sf_t, w1_t, b1_t, tag):
            p = psf.tile([SIDE, B], FP, tag=f"psf{tag}", name="psf")
            nc.tensor.matmul(
                p[:, :], lhsT=w1_t[:FDIM, :SIDE], rhs=sf_t[:FDIM, :B],
                start=True, stop=True,
            )
            fT = res.tile([SIDE, B], BF, tag=f"fT{tag}")
            nc.scalar.activation(
                out=fT[:, :], in_=p[:, :], func=AF.Relu, bias=b1_t[:, :]
            )
            return fT

        # ---- pass 1: masked matmul streams ----
        def pass1(adj_t, w_dram, pspool, nm):
            pst = [
                pspool.tile([128, B], FP, tag=f"ps{nm}{mt}", name=f"ps{nm}")
                for mt in range(2)
            ]
            for g, (k0, nk) in enumerate(GROUPS):
                at = adj_t[g]
                # W arrives in k-pair batches on the sync queue
                wts = {}
                for k2 in range(k0, k0 + nk, 2):
                    wt = wpool.tile([128, 2 * WREC], BF, tag="wt", name="wt")
                    nc.sync.dma_start(
                        out=wt[:, :], in_=w_dram[:, k2 * WREC : (k2 + 2) * WREC]
                    )
                    wts[k2] = wt
                for r in range(R):
                    msk = mpool.tile([128, GMAX], BF, tag="msk", name="msk")
                    nc.vector.tensor_scalar(
                        out=msk[:, : nk * B], in0=at[:, :], scalar1=float(r + 1),
                        scalar2=None, op0=ALU.is_equal,
                    )
                    for j in range(nk):
                        k = k0 + j
                        wt = wts[k & ~1]
                        wof = (k & 1) * WREC + r * M
                        for mt in range(2):
                            nc.tensor.matmul(
                                pst[mt][:, :],
                                lhsT=wt[:, wof + mt * 128 : wof + (mt + 1) * 128],
                                rhs=msk[:, j * B : (j + 1) * B],
                                start=(k == 0 and r == 0),
                                stop=(k == KT - 1 and r == R - 1),
                            )
            return pst

        def evict(pst, nm):
            hT = []
            for mt in range(2):
                h = res.tile([128, B], BF, tag=f"h{nm}T{mt}")
                nc.scalar.activation(out=h[:, :], in_=pst[mt][:, :], func=AF.Relu)
                hT.append(h)
            return hT

        # ---- pass 2: dense head + side head, per 125-row chunk ----
        def pass2(hT, fT, w2t_t, fac_t, o_dram, nm):
            for c in range(4):
                pd = ps2.tile([CH, OUT], FP, tag="pd", name="pd")
                for mt in range(2):
                    nc.tensor.matmul(
                        pd[:, :], lhsT=hT[mt][:, c * CH : (c + 1) * CH],
                        rhs=dwt_t[mt][:, :OUT],
                        start=(mt == 0), stop=(mt == 1),
                    )
                ps_ = ps2.tile([CH, OUT], FP, tag="pss", name="ps_")
                nc.tensor.matmul(
                    ps_[:, :], lhsT=fT[:SIDE, c * CH : (c + 1) * CH],
                    rhs=w2t_t[:SIDE, :OUT], start=True, stop=True,
                )
                sa = scr.tile([CH, OUT], FP, tag="sa", name="sa")
                nc.vector.tensor_scalar(
                    out=sa[:, :], in0=pd[:, :], scalar1=fac_t[:, c : c + 1],
                    scalar2=None, op0=ALU.mult,
                )
                so = scr.tile([CH, OUT], FP, tag="so", name="so")
                nc.vector.tensor_tensor(
                    out=so[:, :], in0=ps_[:, :], in1=sa[:, :], op=ALU.add
                )
                ro = scr.tile([CH, OUT], FP, tag="ro", name="ro")
                nc.scalar.activation(out=ro[:, :], in_=so[:, :], func=AF.Relu)
                nc.scalar.dma_start(
                    out=o_dram[c * CH : (c + 1) * CH, :], in_=ro[:, :]
                )

        # Emission order: side A stream; boundary = side projections + side
        # A pass 2 (overlaps side B's ramp); side B stream; tiny tail.
        pstA = pass1(adjA_t, wi, psA, "A")  # -> Hu^T partials
        huT = evict(pstA, "u")
        fuT = side_proj(uft_t, uw1t_t, ub1_t, "u")
        fvT = side_proj(vft_t, vw1t_t, vb1_t, "v")
        psf.release()
        ps2 = tc.alloc_tile_pool(name="ps2", bufs=2, space="PSUM")
        pass2(huT, fuT, uw2t_t, afac_t, u_out, "u")
        pstB = pass1(adjB_t, wu, psB, "B")  # -> Hv^T partials
        hvT = evict(pstB, "v")
        pass2(hvT, fvT, vw2t_t, bfac_t, v_out, "v")
        ps2.release()

    nc.compile()
    return nc


_CACHE = {}


def _get_program():
    if "nc" not in _CACHE:
        _CACHE["nc"] = build_program()
    return _CACHE["nc"]


def make_in_maps(inputs):
    import ml_dtypes

    bf16 = ml_dtypes.bfloat16

    adj = np.asarray(inputs["adj_matrix"], dtype=np.int32)
    u_sf = np.asarray(inputs["u_sideFeat"], dtype=np.float32)
    v_sf = np.asarray(inputs["v_sideFeat"], dtype=np.float32)
    msg_W = np.asarray(inputs["msg_W"], dtype=np.float64)
    dense_W = np.asarray(inputs["dense_W"], dtype=np.float32)
    u_W1 = np.asarray(inputs["u_W1"], dtype=np.float32)
    u_b1 = np.asarray(inputs["u_b1"], dtype=np.float32).reshape(SIDE, 1)
    u_W2 = np.asarray(inputs["u_W2"], dtype=np.float32)
    v_W1 = np.asarray(inputs["v_W1"], dtype=np.float32)
    v_b1 = np.asarray(inputs["v_b1"], dtype=np.float32).reshape(SIDE, 1)
    v_W2 = np.asarray(inputs["v_W2"], dtype=np.float32)

    # degree normalization (exact, f64); Csafe guard only matters off-support
    nz = adj != 0
    a = 1.0 / np.sqrt(np.maximum(nz.sum(axis=1), 1))  # [NU]
    b = 1.0 / np.sqrt(np.maximum(nz.sum(axis=0), 1))  # [NI]

    # deg-folded transposed message weights, zero-padded to NP, then laid
    # out partition-major in PE stream order: [p, k*WREC + r*M + m]
    def w_stream(wT):  # wT [R, 4000, M] -> [128, KT*WREC]
        wp = np.zeros((R, NP, M), np.float64)
        wp[:, :NU, :] = wT
        return np.ascontiguousarray(
            wp.reshape(R, KT, 128, M)
            .transpose(2, 1, 0, 3)
            .reshape(128, KT * WREC)
        ).astype(bf16)

    wi_s = w_stream(msg_W[:, :, NU:].transpose(0, 2, 1) * b[None, :, None])
    wu_s = w_stream(msg_W[:, :, :NU].transpose(0, 2, 1) * a[None, :, None])

    # adjacency blocks, padded, partition-major: [p, k*B + u]
    def adj_stream(m):  # m [4000(rows->contraction), 4000(cols)] -> per-core
        mp = np.zeros((NP, NU), np.float32)
        mp[:NU, :] = m
        return mp.reshape(KT, 128, NU).transpose(1, 0, 2)  # [128, KT, 4000]

    adjv_s = adj_stream(adj)  # contraction over users (rows)
    adjtu_s = adj_stream(adj.T)  # contraction over items (rows of adj^T)
    uftT = np.ascontiguousarray(u_sf.T.astype(bf16))
    vftT = np.ascontiguousarray(v_sf.T.astype(bf16))

    def chunked(v):  # [B] f64 -> [CH, 4] f32 column-per-chunk
        return np.ascontiguousarray(v.reshape(4, CH).T).astype(np.float32)

    shared = {
        "wi": wi_s,
        "wu": wu_s,
        "uw1t": np.ascontiguousarray(u_W1.T).astype(bf16),
        "vw1t": np.ascontiguousarray(v_W1.T).astype(bf16),
        "ub1": u_b1,
        "vb1": v_b1,
        "uw2t": np.ascontiguousarray(u_W2.T).astype(bf16),
        "vw2t": np.ascontiguousarray(v_W2.T).astype(bf16),
        "dwt": np.ascontiguousarray(dense_W.T).astype(bf16),
    }
    in_maps = []
    for c in range(NCORES):
        s = c * B
        in_maps.append(
            {
                **shared,
                "adjtu": np.ascontiguousarray(
                    adjtu_s[:, :, s : s + B].reshape(128, KT * B)
                ).astype(bf16),
                "adjv": np.ascontiguousarray(
                    adjv_s[:, :, s : s + B].reshape(128, KT * B)
                ).astype(bf16),
                "uft": np.ascontiguousarray(uftT[:, s : s + B]),
                "vft": np.ascontiguousarray(vftT[:, s : s + B]),
                "afac": chunked(a[s : s + B]),
                "bfac": chunked(b[s : s + B]),
            }
        )
    return in_maps


def assemble(results):
    U = np.empty((NU, OUT), np.float32)
    V = np.empty((NI, OUT), np.float32)
    for c in range(NCORES):
        U[c * B : (c + 1) * B] = results[c]["u_out"]
        V[c * B : (c + 1) * B] = results[c]["v_out"]
    return (U, V)


def kernel(**inputs):
    from concourse.bass_utils import run_bass_kernel_spmd

    nc = _get_program()
    res = run_bass_kernel_spmd(nc, make_in_maps(inputs), core_ids=list(range(NCORES)))
    return assemble(res.results)


# revision 26
# speedup vs baseline: 1.0427x; 1.0427x over previous
"""Trainium2 Bass kernel for the bipartite GNN message-passing encoder.

Math:
  A_r = (adj == r), r = 1..5
  An_r = diag(a) A_r diag(b),  a = 1/sqrt(Nu), b = 1/sqrt(Nv)
  Hu = relu(sum_r An_r @ W_items_r^T)   [NU, M]
  Hv = relu(sum_r An_r^T @ W_users_r^T) [NI, M]
  U  = relu(Hu @ dense_W^T + relu(u_sf @ u_W1^T + u_b1) @ u_W2^T)
  V  = relu(Hv @ dense_W^T + relu(v_sf @ v_W1^T + v_b1) @ v_W2^T)

Sharding: fully collective-free 1D row split per bipartite side. Core c
owns users [500c, 500c+500) and items [500c, 500c+500) and contracts
over the FULL opposite side locally, so no partial-sum AllReduce (and no
barrier) is ever needed. Degree normalizations are folded in on the
host: b into the item-message weights, a into the user-message weights;
the outer-side factor commutes through the relu (a,b > 0) and is applied
as a per-partition scale in pass 2.

Device program = one back-to-back bf16 matmul stream (the PE is the
bottleneck at ~142us; HAM holds 2.4 GHz only while the stream is dense).
All operands arrive pre-transposed/pre-converted in consumption order.
k-tiles are processed in GROUPS laid out contiguously per partition so
one DMA loads a whole group and one DVE is_equal builds a whole group's
rating mask -- few DMAs/DVE ops means no HW-semaphore-slot convoys
(8 slots shared by all queues; many small DMAs with distant consumers
stall the just-in-time W stream). Ramp groups are small so the first
matmul issues ~3us in. Side A's pass 2 + both side-feature projections
sit at the A->B boundary; only side B's tiny pass 2 trails the stream.
"""

import sys

import numpy as np

if "/opt/trn_rl_repo" not in sys.path:
    sys.path.insert(0, "/opt/trn_rl_repo")

import concourse.bacc as bacc  # noqa: E402
import concourse.mybir as mybir  # noqa: E402
import concourse.tile as tile  # noqa: E402

FP = mybir.dt.float32
BF = mybir.dt.bfloat16

NU = NI = 4000
R = 5
M = 256
OUT = 75
SIDE = 64
FDIM = 128

NCORES = 8
B = NU // NCORES  # 500 rows per side per core
NP = 4096  # contraction dim padded to a multiple of 128 (pad rows are
#            adj=0 => every rating mask is 0 there, contributing nothing)
KT = NP // 128  # 32 contraction k-tiles (before splitting by rating)
CH = B // 4  # 125-row output chunks in pass 2
WREC = R * M  # 1280 W columns per k-tile

# k-tile groups: one adj DMA + one mask per (group, r); small ramp first
GROUPS = [(0, 2), (2, 2), (4, 4), (8, 8), (16, 8), (24, 8)]
GMAX = 8 * B  # widest group mask

AF = mybir.ActivationFunctionType
ALU = mybir.AluOpType


def build_program():
    from contextlib import ExitStack

    nc = bacc.Bacc("TRN2", target_bir_lowering=False, debug=False, num_devices=NCORES)

    # ---- I/O. adj streams: [p, k*B + u] = adj_kblock[k][p, u]; W streams:
    # [p, k*WREC + r*M + m] = W^T[r, 128k+p, m] (deg-folded). ----
    adjtu = nc.dram_tensor("adjtu", [128, KT * B], BF, kind="ExternalInput")
    adjv = nc.dram_tensor("adjv", [128, KT * B], BF, kind="ExternalInput")
    wi = nc.dram_tensor("wi", [128, KT * WREC], BF, kind="ExternalInput")
    wu = nc.dram_tensor("wu", [128, KT * WREC], BF, kind="ExternalInput")
    uft = nc.dram_tensor("uft", [FDIM, B], BF, kind="ExternalInput")
    vft = nc.dram_tensor("vft", [FDIM, B], BF, kind="ExternalInput")
    uw1t = nc.dram_tensor("uw1t", [FDIM, SIDE], BF, kind="ExternalInput")
    vw1t = nc.dram_tensor("vw1t", [FDIM, SIDE], BF, kind="ExternalInput")
    ub1 = nc.dram_tensor("ub1", [SIDE, 1], FP, kind="ExternalInput")
    vb1 = nc.dram_tensor("vb1", [SIDE, 1], FP, kind="ExternalInput")
    uw2t = nc.dram_tensor("uw2t", [SIDE, OUT], BF, kind="ExternalInput")
    vw2t = nc.dram_tensor("vw2t", [SIDE, OUT], BF, kind="ExternalInput")
    dwt = nc.dram_tensor("dwt", [M, OUT], BF, kind="ExternalInput")
    afac = nc.dram_tensor("afac", [CH, 4], FP, kind="ExternalInput")
    bfac = nc.dram_tensor("bfac", [CH, 4], FP, kind="ExternalInput")
    u_out = nc.dram_tensor("u_out", [B, OUT], FP, kind="ExternalOutput")
    v_out = nc.dram_tensor("v_out", [B, OUT], FP, kind="ExternalOutput")

    with tile.TileContext(nc) as tc, ExitStack() as ctx:
        res = ctx.enter_context(tc.tile_pool(name="res", bufs=1))
        wpool = ctx.enter_context(tc.tile_pool(name="wpool", bufs=8))
        apool = ctx.enter_context(tc.tile_pool(name="apool", bufs=3))
        mpool = ctx.enter_context(tc.tile_pool(name="mpool", bufs=4))
        scr = ctx.enter_context(tc.tile_pool(name="scr", bufs=3))
        # PSUM budget (8 banks): psA 2 + psB 2 + psf 2 (released before
        # pass 2 opens ps2's 4)
        psA = ctx.enter_context(tc.tile_pool(name="psA", bufs=1, space="PSUM"))
        psB = ctx.enter_context(tc.tile_pool(name="psB", bufs=1, space="PSUM"))
        psf = tc.alloc_tile_pool(name="psf", bufs=1, space="PSUM")

        # ---- small resident loads (ACT queue; all are first needed at the
        # A->B boundary or later) ----
        def rload(dram, p, f, dt, tag):
            t = res.tile([p, f], dt, tag=tag)
            nc.scalar.dma_start(out=t[:, :], in_=dram[:, :])
            return t

        uft_t = rload(uft, FDIM, B, BF, "uft")
        vft_t = rload(vft, FDIM, B, BF, "vft")
        uw1t_t = rload(uw1t, FDIM, SIDE, BF, "uw1t")
        vw1t_t = rload(vw1t, FDIM, SIDE, BF, "vw1t")
        ub1_t = rload(ub1, SIDE, 1, FP, "ub1")
        vb1_t = rload(vb1, SIDE, 1, FP, "vb1")
        uw2t_t = rload(uw2t, SIDE, OUT, BF, "uw2t")
        vw2t_t = rload(vw2t, SIDE, OUT, BF, "vw2t")
        afac_t = rload(afac, CH, 4, FP, "afac")
        bfac_t = rload(bfac, CH, 4, FP, "bfac")
        dwt_t = []
        for mt in range(2):
            t = res.tile([128, OUT], BF, tag=f"dwt{mt}")
            nc.scalar.dma_start(out=t[:, :], in_=dwt[mt * 128 : (mt + 1) * 128, :])
            dwt_t.append(t)

        # ---- side-feature projections (PE-tiny; emitted at the boundary) ---
        def side_proj(sf_t, w1_t, b1_t, tag):
            p = psf.tile([SIDE, B], FP, tag=f"psf{tag}", name="psf")
            nc.tensor.matmul(
                p[:, :], lhsT=w1_t[:FDIM, :SIDE], rhs=sf_t[:FDIM, :B],
                start=True, stop=True,
            )
            fT = res.tile([SIDE, B], BF, tag=f"fT{tag}")
            nc.scalar.activation(
                out=fT[:, :], in_=p[:, :], func=AF.Relu, bias=b1_t[:, :]
            )
            return fT

        # ---- pass 1: masked matmul streams. adj group DMAs ride a small
        # ring (ACT queue) so they're paced by mask consumption instead of
        # front-loading 8MB against the just-in-time W stream. ----
        def pass1(adj_dram, w_dram, pspool, nm):
            pst = [
                pspool.tile([128, B], FP, tag=f"ps{nm}{mt}", name=f"ps{nm}")
                for mt in range(2)
            ]
            for g, (k0, nk) in enumerate(GROUPS):
                at = apool.tile([128, GMAX], BF, tag="adjg", name="at")
                nc.scalar.dma_start(
                    out=at[:, : nk * B], in_=adj_dram[:, k0 * B : (k0 + nk) * B]
                )
                # W arrives in k-pair batches on the sync queue
                wts = {}
                for k2 in range(k0, k0 + nk, 2):
                    wt = wpool.tile([128, 2 * WREC], BF, tag="wt", name="wt")
                    nc.sync.dma_start(
                        out=wt[:, :], in_=w_dram[:, k2 * WREC : (k2 + 2) * WREC]
                    )
                    wts[k2] = wt
                for r in range(R):
                    msk = mpool.tile([128, GMAX], BF, tag="msk", name="msk")
                    nc.vector.tensor_scalar(
                        out=msk[:, : nk * B], in0=at[:, : nk * B],
                        scalar1=float(r + 1), scalar2=None, op0=ALU.is_equal,
                    )
                    for j in range(nk):
                        k = k0 + j
                        wt = wts[k & ~1]
                        wof = (k & 1) * WREC + r * M
                        for mt in range(2):
                            nc.tensor.matmul(
                                pst[mt][:, :],
                                lhsT=wt[:, wof + mt * 128 : wof + (mt + 1) * 128],
                                rhs=msk[:, j * B : (j + 1) * B],
                                start=(k == 0 and r == 0),
                                stop=(k == KT - 1 and r == R - 1),
                            )
            return pst

        def evict(pst, nm):
            hT = []
            for mt in range(2):
                h = res.tile([128, B], BF, tag=f"h{nm}T{mt}")
                nc.scalar.activation(out=h[:, :], in_=pst[mt][:, :], func=AF.Relu)
                hT.append(h)
            return hT

        # ---- pass 2: dense head + side head, per 125-row chunk ----
        def pass2(hT, fT, w2t_t, fac_t, o_dram, nm):
            for c in range(4):
                pd = ps2.tile([CH, OUT], FP, tag="pd", name="pd")
                for mt in range(2):
                    nc.tensor.matmul(
                        pd[:, :], lhsT=hT[mt][:, c * CH : (c + 1) * CH],
                        rhs=dwt_t[mt][:, :OUT],
                        start=(mt == 0), stop=(mt == 1),
                    )
                ps_ = ps2.tile([CH, OUT], FP, tag="pss", name="ps_")
                nc.tensor.matmul(
                    ps_[:, :], lhsT=fT[:SIDE, c * CH : (c + 1) * CH],
                    rhs=w2t_t[:SIDE, :OUT], start=True, stop=True,
                )
                sa = scr.tile([CH, OUT], FP, tag="sa", name="sa")
                nc.vector.tensor_scalar(
                    out=sa[:, :], in0=pd[:, :], scalar1=fac_t[:, c : c + 1],
                    scalar2=None, op0=ALU.mult,
                )
                so = scr.tile([CH, OUT], FP, tag="so", name="so")
                nc.vector.tensor_tensor(
                    out=so[:, :], in0=ps_[:, :], in1=sa[:, :], op=ALU.add
                )
                ro = scr.tile([CH, OUT], FP, tag="ro", name="ro")
                nc.scalar.activation(out=ro[:, :], in_=so[:, :], func=AF.Relu)
                nc.scalar.dma_start(
                    out=o_dram[c * CH : (c + 1) * CH, :], in_=ro[:, :]
                )

        # Emission order: side A stream; boundary = side projections + side
        # A pass 2 (overlaps side B's ramp); side B stream; tiny tail.
        pstA = pass1(adjtu, wi, psA, "A")  # -> Hu^T partials
        huT = evict(pstA, "u")
        fuT = side_proj(uft_t, uw1t_t, ub1_t, "u")
        fvT = side_proj(vft_t, vw1t_t, vb1_t, "v")
        psf.release()
        ps2 = tc.alloc_tile_pool(name="ps2", bufs=2, space="PSUM")
        pass2(huT, fuT, uw2t_t, afac_t, u_out, "u")
        pstB = pass1(adjv, wu, psB, "B")  # -> Hv^T partials
        hvT = evict(pstB, "v")
        pass2(hvT, fvT, vw2t_t, bfac_t, v_out, "v")
        ps2.release()

    nc.compile()
    return nc


_CACHE = {}


def _get_program():
    if "nc" not in _CACHE:
        _CACHE["nc"] = build_program()
    return _CACHE["nc"]


def make_in_maps(inputs):
    import ml_dtypes

    bf16 = ml_dtypes.bfloat16

    adj = np.asarray(inputs["adj_matrix"], dtype=np.int32)
    u_sf = np.asarray(inputs["u_sideFeat"], dtype=np.float32)
    v_sf = np.asarray(inputs["v_sideFeat"], dtype=np.float32)
    msg_W = np.asarray(inputs["msg_W"], dtype=np.float64)
    dense_W = np.asarray(inputs["dense_W"], dtype=np.float32)
    u_W1 = np.asarray(inputs["u_W1"], dtype=np.float32)
    u_b1 = np.asarray(inputs["u_b1"], dtype=np.float32).reshape(SIDE, 1)
    u_W2 = np.asarray(inputs["u_W2"], dtype=np.float32)
    v_W1 = np.asarray(inputs["v_W1"], dtype=np.float32)
    v_b1 = np.asarray(inputs["v_b1"], dtype=np.float32).reshape(SIDE, 1)
    v_W2 = np.asarray(inputs["v_W2"], dtype=np.float32)

    # degree normalization (exact, f64); Csafe guard only matters off-support
    nz = adj != 0
    a = 1.0 / np.sqrt(np.maximum(nz.sum(axis=1), 1))  # [NU]
    b = 1.0 / np.sqrt(np.maximum(nz.sum(axis=0), 1))  # [NI]

    # deg-folded transposed message weights, zero-padded to NP, then laid
    # out partition-major in PE stream order: [p, k*WREC + r*M + m]
    def w_stream(wT):  # wT [R, 4000, M] -> [128, KT*WREC]
        wp = np.zeros((R, NP, M), np.float64)
        wp[:, :NU, :] = wT
        return np.ascontiguousarray(
            wp.reshape(R, KT, 128, M)
            .transpose(2, 1, 0, 3)
            .reshape(128, KT * WREC)
        ).astype(bf16)

    wi_s = w_stream(msg_W[:, :, NU:].transpose(0, 2, 1) * b[None, :, None])
    wu_s = w_stream(msg_W[:, :, :NU].transpose(0, 2, 1) * a[None, :, None])

    # adjacency blocks, padded, partition-major: [p, k*B + u]
    def adj_stream(m):  # m [4000(rows->contraction), 4000(cols)] -> per-core
        mp = np.zeros((NP, NU), np.float32)
        mp[:NU, :] = m
        return mp.reshape(KT, 128, NU).transpose(1, 0, 2)  # [128, KT, 4000]

    adjv_s = adj_stream(adj)  # contraction over users (rows)
    adjtu_s = adj_stream(adj.T)  # contraction over items (rows of adj^T)
    uftT = np.ascontiguousarray(u_sf.T.astype(bf16))
    vftT = np.ascontiguousarray(v_sf.T.astype(bf16))

    def chunked(v):  # [B] f64 -> [CH, 4] f32 column-per-chunk
        return np.ascontiguousarray(v.reshape(4, CH).T).astype(np.float32)

    shared = {
        "wi": wi_s,
        "wu": wu_s,
        "uw1t": np.ascontiguousarray(u_W1.T).astype(bf16),
        "vw1t": np.ascontiguousarray(v_W1.T).astype(bf16),
        "ub1": u_b1,
        "vb1": v_b1,
        "uw2t": np.ascontiguousarray(u_W2.T).astype(bf16),
        "vw2t": np.ascontiguousarray(v_W2.T).astype(bf16),
        "dwt": np.ascontiguousarray(dense_W.T).astype(bf16),
    }
    in_maps = []
    for c in range(NCORES):
        s = c * B
        in_maps.append(
            {
                **shared,
                "adjtu": np.ascontiguousarray(
                    adjtu_s[:, :, s : s + B].reshape(128, KT * B)
                ).astype(bf16),
                "adjv": np.ascontiguousarray(
                    adjv_s[:, :, s : s + B].reshape(128, KT * B)
                ).astype(bf16),
                "uft": np.ascontiguousarray(uftT[:, s : s + B]),
                "vft": np.ascontiguousarray(vftT[:, s : s + B]),
                "afac": chunked(a[s : s + B]),
                "bfac": chunked(b[s : s + B]),
            }
        )
    return in_maps


def assemble(results):
    U = np.empty((NU, OUT), np.float32)
    V = np.empty((NI, OUT), np.float32)
    for c in range(NCORES):
        U[c * B : (c + 1) * B] = results[c]["u_out"]
        V[c * B : (c + 1) * B] = results[c]["v_out"]
    return (U, V)


def kernel(**inputs):
    from concourse.bass_utils import run_bass_kernel_spmd

    nc = _get_program()
    res = run_bass_kernel_spmd(nc, make_in_maps(inputs), core_ids=list(range(NCORES)))
    return assemble(res.results)


# revision 31
# speedup vs baseline: 1.0896x; 1.0450x over previous
"""Trainium2 Bass kernel for the bipartite GNN message-passing encoder.

Math:
  A_r = (adj == r), r = 1..5
  An_r = diag(a) A_r diag(b),  a = 1/sqrt(Nu), b = 1/sqrt(Nv)
  Hu = relu(sum_r An_r @ W_items_r^T)   [NU, M]
  Hv = relu(sum_r An_r^T @ W_users_r^T) [NI, M]
  U  = relu(Hu @ dense_W^T + relu(u_sf @ u_W1^T + u_b1) @ u_W2^T)
  V  = relu(Hv @ dense_W^T + relu(v_sf @ v_W1^T + v_b1) @ v_W2^T)

Sharding: fully collective-free 1D row split per bipartite side. Core c
owns users [500c, 500c+500) and items [500c, 500c+500) and contracts
over the FULL opposite side locally, so no partial-sum AllReduce (and no
barrier) is ever needed. Degree normalizations are folded in on the
host: b into the item-message weights, a into the user-message weights;
the outer-side factor commutes through the relu (a,b > 0) and is applied
as a per-partition scale in pass 2.

Device program = one back-to-back bf16 matmul stream (the PE is the
bottleneck at ~142us; HAM holds 2.4 GHz only while the stream is dense).
All operands arrive pre-transposed/pre-converted in consumption order.
k-tiles are processed in GROUPS laid out contiguously per partition so
one DMA loads a whole group and one DVE is_equal builds a whole group's
rating mask -- few DMAs/DVE ops means no HW-semaphore-slot convoys
(8 slots shared by all queues; many small DMAs with distant consumers
stall the just-in-time W stream). Ramp groups are small so the first
matmul issues ~3us in. Side A's pass 2 + both side-feature projections
sit at the A->B boundary; only side B's tiny pass 2 trails the stream.
"""

import sys

import numpy as np

if "/opt/trn_rl_repo" not in sys.path:
    sys.path.insert(0, "/opt/trn_rl_repo")

import concourse.bacc as bacc  # noqa: E402
import concourse.mybir as mybir  # noqa: E402
import concourse.tile as tile  # noqa: E402

FP = mybir.dt.float32
BF = mybir.dt.bfloat16

NU = NI = 4000
R = 5
M = 256
OUT = 75
SIDE = 64
FDIM = 128

NCORES = 8
B = NU // NCORES  # 500 rows per side per core
NP = 4096  # contraction dim padded to a multiple of 128 (pad rows are
#            adj=0 => every rating mask is 0 there, contributing nothing)
KT = NP // 128  # 32 contraction k-tiles (before splitting by rating)
CH = B // 4  # 125-row output chunks in pass 2
WREC = R * M  # 1280 W columns per k-tile

# k-tile groups: one adj DMA + one mask per (group, r); small ramp first
GROUPS = [(0, 1), (1, 1), (2, 2), (4, 4), (8, 8), (16, 8), (24, 8)]
GMAX = 8 * B  # widest group mask

AF = mybir.ActivationFunctionType
ALU = mybir.AluOpType


def build_program():
    from contextlib import ExitStack

    nc = bacc.Bacc("TRN2", target_bir_lowering=False, debug=False, num_devices=NCORES)

    # ---- I/O. adj streams: [p, k*B + u] = adj_kblock[k][p, u]; W streams:
    # [p, k*WREC + r*M + m] = W^T[r, 128k+p, m] (deg-folded). ----
    adjtu = nc.dram_tensor("adjtu", [128, KT * B], BF, kind="ExternalInput")
    adjv = nc.dram_tensor("adjv", [128, KT * B], BF, kind="ExternalInput")
    wi = nc.dram_tensor("wi", [128, KT * WREC], BF, kind="ExternalInput")
    wu = nc.dram_tensor("wu", [128, KT * WREC], BF, kind="ExternalInput")
    uft = nc.dram_tensor("uft", [FDIM, B], BF, kind="ExternalInput")
    vft = nc.dram_tensor("vft", [FDIM, B], BF, kind="ExternalInput")
    uw1t = nc.dram_tensor("uw1t", [FDIM, SIDE], BF, kind="ExternalInput")
    vw1t = nc.dram_tensor("vw1t", [FDIM, SIDE], BF, kind="ExternalInput")
    ub1 = nc.dram_tensor("ub1", [SIDE, 1], FP, kind="ExternalInput")
    vb1 = nc.dram_tensor("vb1", [SIDE, 1], FP, kind="ExternalInput")
    uw2t = nc.dram_tensor("uw2t", [SIDE, OUT], BF, kind="ExternalInput")
    vw2t = nc.dram_tensor("vw2t", [SIDE, OUT], BF, kind="ExternalInput")
    dwt = nc.dram_tensor("dwt", [M, OUT], BF, kind="ExternalInput")
    afac = nc.dram_tensor("afac", [CH, 4], FP, kind="ExternalInput")
    bfac = nc.dram_tensor("bfac", [CH, 4], FP, kind="ExternalInput")
    u_out = nc.dram_tensor("u_out", [B, OUT], FP, kind="ExternalOutput")
    v_out = nc.dram_tensor("v_out", [B, OUT], FP, kind="ExternalOutput")

    with tile.TileContext(nc) as tc, ExitStack() as ctx:
        res = ctx.enter_context(tc.tile_pool(name="res", bufs=1))
        wpool = ctx.enter_context(tc.tile_pool(name="wpool", bufs=8))
        apool = ctx.enter_context(tc.tile_pool(name="apool", bufs=3))
        mpool = ctx.enter_context(tc.tile_pool(name="mpool", bufs=4))
        scr = ctx.enter_context(tc.tile_pool(name="scr", bufs=3))
        # PSUM budget (8 banks): psA 2 + psB 2 + psf 2 (released before
        # pass 2 opens ps2's 4)
        psA = ctx.enter_context(tc.tile_pool(name="psA", bufs=1, space="PSUM"))
        psB = ctx.enter_context(tc.tile_pool(name="psB", bufs=1, space="PSUM"))
        psf = tc.alloc_tile_pool(name="psf", bufs=1, space="PSUM")

        # ---- small resident loads (ACT queue; emitted after side A's pass 1
        # so the first adj group is the very first scalar-queue DMA; all of
        # these are first needed at the A->B boundary or later) ----
        def rload(dram, p, f, dt, tag):
            t = res.tile([p, f], dt, tag=tag)
            nc.scalar.dma_start(out=t[:, :], in_=dram[:, :])
            return t

        def load_smalls():
            sm = {
                "uft": rload(uft, FDIM, B, BF, "uft"),
                "vft": rload(vft, FDIM, B, BF, "vft"),
                "uw1t": rload(uw1t, FDIM, SIDE, BF, "uw1t"),
                "vw1t": rload(vw1t, FDIM, SIDE, BF, "vw1t"),
                "ub1": rload(ub1, SIDE, 1, FP, "ub1"),
                "vb1": rload(vb1, SIDE, 1, FP, "vb1"),
                "uw2t": rload(uw2t, SIDE, OUT, BF, "uw2t"),
                "vw2t": rload(vw2t, SIDE, OUT, BF, "vw2t"),
                "afac": rload(afac, CH, 4, FP, "afac"),
                "bfac": rload(bfac, CH, 4, FP, "bfac"),
            }
            dwt_t = []
            for mt in range(2):
                t = res.tile([128, OUT], BF, tag=f"dwt{mt}")
                nc.scalar.dma_start(
                    out=t[:, :], in_=dwt[mt * 128 : (mt + 1) * 128, :]
                )
                dwt_t.append(t)
            sm["dwt"] = dwt_t
            return sm

        # ---- side-feature projections (PE-tiny; emitted at the boundary) ---
        def side_proj(sf_t, w1_t, b1_t, tag):
            p = psf.tile([SIDE, B], FP, tag=f"psf{tag}", name="psf")
            nc.tensor.matmul(
                p[:, :], lhsT=w1_t[:FDIM, :SIDE], rhs=sf_t[:FDIM, :B],
                start=True, stop=True,
            )
            fT = res.tile([SIDE, B], BF, tag=f"fT{tag}")
            nc.scalar.activation(
                out=fT[:, :], in_=p[:, :], func=AF.Relu, bias=b1_t[:, :]
            )
            return fT

        # ---- pass 1: masked matmul streams. adj group DMAs ride a small
        # ring (ACT queue) so they're paced by mask consumption instead of
        # front-loading 8MB against the just-in-time W stream. ----
        def pass1(adj_dram, w_dram, pspool, nm):
            pst = [
                pspool.tile([128, B], FP, tag=f"ps{nm}{mt}", name=f"ps{nm}")
                for mt in range(2)
            ]
            wts = {}  # k -> (tile, col offset); ramp groups load single k
            for g, (k0, nk) in enumerate(GROUPS):
                at = apool.tile([128, GMAX], BF, tag="adjg", name="at")
                nc.scalar.dma_start(
                    out=at[:, : nk * B], in_=adj_dram[:, k0 * B : (k0 + nk) * B]
                )
                # W arrives on the sync queue: single-k for the ramp groups,
                # k-pair batches in steady state
                if nk == 1:
                    wt = wpool.tile([128, WREC], BF, tag="wt1", bufs=2, name="wt")
                    nc.sync.dma_start(
                        out=wt[:, :], in_=w_dram[:, k0 * WREC : (k0 + 1) * WREC]
                    )
                    wts[k0] = (wt, 0)
                else:
                    for k2 in range(k0, k0 + nk, 2):
                        wt = wpool.tile([128, 2 * WREC], BF, tag="wt", name="wt")
                        nc.sync.dma_start(
                            out=wt[:, :], in_=w_dram[:, k2 * WREC : (k2 + 2) * WREC]
                        )
                        wts[k2] = (wt, 0)
                        wts[k2 + 1] = (wt, WREC)
                for r in range(R):
                    msk = mpool.tile([128, GMAX], BF, tag="msk", name="msk")
                    nc.vector.tensor_scalar(
                        out=msk[:, : nk * B], in0=at[:, : nk * B],
                        scalar1=float(r + 1), scalar2=None, op0=ALU.is_equal,
                    )
                    for j in range(nk):
                        k = k0 + j
                        wt, base = wts[k]
                        wof = base + r * M
                        for mt in range(2):
                            nc.tensor.matmul(
                                pst[mt][:, :],
                                lhsT=wt[:, wof + mt * 128 : wof + (mt + 1) * 128],
                                rhs=msk[:, j * B : (j + 1) * B],
                                start=(k == 0 and r == 0),
                                stop=(k == KT - 1 and r == R - 1),
                            )
            return pst

        def evict(pst, nm):
            hT = []
            for mt in range(2):
                h = res.tile([128, B], BF, tag=f"h{nm}T{mt}")
                nc.scalar.activation(out=h[:, :], in_=pst[mt][:, :], func=AF.Relu)
                hT.append(h)
            return hT

        # ---- pass 2: dense head + side head, per 125-row chunk ----
        def pass2(hT, fT, w2t_t, fac_t, dwt_t, o_dram, nm):
            for c in range(4):
                pd = ps2.tile([CH, OUT], FP, tag="pd", name="pd")
                for mt in range(2):
                    nc.tensor.matmul(
                        pd[:, :], lhsT=hT[mt][:, c * CH : (c + 1) * CH],
                        rhs=dwt_t[mt][:, :OUT],
                        start=(mt == 0), stop=(mt == 1),
                    )
                ps_ = ps2.tile([CH, OUT], FP, tag="pss", name="ps_")
                nc.tensor.matmul(
                    ps_[:, :], lhsT=fT[:SIDE, c * CH : (c + 1) * CH],
                    rhs=w2t_t[:SIDE, :OUT], start=True, stop=True,
                )
                sa = scr.tile([CH, OUT], FP, tag="sa", name="sa")
                nc.vector.tensor_scalar(
                    out=sa[:, :], in0=pd[:, :], scalar1=fac_t[:, c : c + 1],
                    scalar2=None, op0=ALU.mult,
                )
                so = scr.tile([CH, OUT], FP, tag="so", name="so")
                nc.vector.tensor_tensor(
                    out=so[:, :], in0=ps_[:, :], in1=sa[:, :], op=ALU.add
                )
                ro = scr.tile([CH, OUT], FP, tag="ro", name="ro")
                nc.scalar.activation(out=ro[:, :], in_=so[:, :], func=AF.Relu)
                nc.scalar.dma_start(
                    out=o_dram[c * CH : (c + 1) * CH, :], in_=ro[:, :]
                )

        # PE keep-alive: dummy matmuls that hold the HAM clock at full speed
        # while the short ACT/DVE pass-2 chains drain (results never read)
        def dummy_mms(n, src):
            for _ in range(n):
                t = psA.tile([128, B], FP, tag="psA0", name="dmy")
                nc.tensor.matmul(
                    t[:, :], lhsT=src[:, :128], rhs=src[:, :B],
                    start=True, stop=True, skip_group_check=True,
                )

        # Emission order: side A stream; boundary = side projections + side
        # A pass 2 (overlaps side B's ramp); side B stream; tiny tail.
        pstA = pass1(adjtu, wi, psA, "A")  # -> Hu^T partials
        huT = evict(pstA, "u")
        sm = load_smalls()
        fuT = side_proj(sm["uft"], sm["uw1t"], sm["ub1"], "u")
        fvT = side_proj(sm["vft"], sm["vw1t"], sm["vb1"], "v")
        psf.release()
        ps2 = tc.alloc_tile_pool(name="ps2", bufs=2, space="PSUM")
        pass2(huT, fuT, sm["uw2t"], sm["afac"], sm["dwt"], u_out, "u")
        pstB = pass1(adjv, wu, psB, "B")  # -> Hv^T partials
        hvT = evict(pstB, "v")
        dummy_mms(8, huT[0])
        pass2(hvT, fvT, sm["vw2t"], sm["bfac"], sm["dwt"], v_out, "v")
        dummy_mms(10, huT[0])
        ps2.release()

    nc.compile()
    return nc


_CACHE = {}


def _get_program():
    if "nc" not in _CACHE:
        _CACHE["nc"] = build_program()
    return _CACHE["nc"]


def make_in_maps(inputs):
    import ml_dtypes

    bf16 = ml_dtypes.bfloat16

    adj = np.asarray(inputs["adj_matrix"], dtype=np.int32)
    u_sf = np.asarray(inputs["u_sideFeat"], dtype=np.float32)
    v_sf = np.asarray(inputs["v_sideFeat"], dtype=np.float32)
    msg_W = np.asarray(inputs["msg_W"], dtype=np.float64)
    dense_W = np.asarray(inputs["dense_W"], dtype=np.float32)
    u_W1 = np.asarray(inputs["u_W1"], dtype=np.float32)
    u_b1 = np.asarray(inputs["u_b1"], dtype=np.float32).reshape(SIDE, 1)
    u_W2 = np.asarray(inputs["u_W2"], dtype=np.float32)
    v_W1 = np.asarray(inputs["v_W1"], dtype=np.float32)
    v_b1 = np.asarray(inputs["v_b1"], dtype=np.float32).reshape(SIDE, 1)
    v_W2 = np.asarray(inputs["v_W2"], dtype=np.float32)

    # degree normalization (exact, f64); Csafe guard only matters off-support
    nz = adj != 0
    a = 1.0 / np.sqrt(np.maximum(nz.sum(axis=1), 1))  # [NU]
    b = 1.0 / np.sqrt(np.maximum(nz.sum(axis=0), 1))  # [NI]

    # deg-folded transposed message weights, zero-padded to NP, then laid
    # out partition-major in PE stream order: [p, k*WREC + r*M + m]
    def w_stream(wT):  # wT [R, 4000, M] -> [128, KT*WREC]
        wp = np.zeros((R, NP, M), np.float64)
        wp[:, :NU, :] = wT
        return np.ascontiguousarray(
            wp.reshape(R, KT, 128, M)
            .transpose(2, 1, 0, 3)
            .reshape(128, KT * WREC)
        ).astype(bf16)

    wi_s = w_stream(msg_W[:, :, NU:].transpose(0, 2, 1) * b[None, :, None])
    wu_s = w_stream(msg_W[:, :, :NU].transpose(0, 2, 1) * a[None, :, None])

    # adjacency blocks, padded, partition-major: [p, k*B + u]
    def adj_stream(m):  # m [4000(rows->contraction), 4000(cols)] -> per-core
        mp = np.zeros((NP, NU), np.float32)
        mp[:NU, :] = m
        return mp.reshape(KT, 128, NU).transpose(1, 0, 2)  # [128, KT, 4000]

    adjv_s = adj_stream(adj)  # contraction over users (rows)
    adjtu_s = adj_stream(adj.T)  # contraction over items (rows of adj^T)
    uftT = np.ascontiguousarray(u_sf.T.astype(bf16))
    vftT = np.ascontiguousarray(v_sf.T.astype(bf16))

    def chunked(v):  # [B] f64 -> [CH, 4] f32 column-per-chunk
        return np.ascontiguousarray(v.reshape(4, CH).T).astype(np.float32)

    shared = {
        "wi": wi_s,
        "wu": wu_s,
        "uw1t": np.ascontiguousarray(u_W1.T).astype(bf16),
        "vw1t": np.ascontiguousarray(v_W1.T).astype(bf16),
        "ub1": u_b1,
        "vb1": v_b1,
        "uw2t": np.ascontiguousarray(u_W2.T).astype(bf16),
        "vw2t": np.ascontiguousarray(v_W2.T).astype(bf16),
        "dwt": np.ascontiguousarray(dense_W.T).astype(bf16),
    }
    in_maps = []
    for c in range(NCORES):
        s = c * B
        in_maps.append(
            {
                **shared,
                "adjtu": np.ascontiguousarray(
                    adjtu_s[:, :, s : s + B].reshape(128, KT * B)
                ).astype(bf16),
                "adjv": np.ascontiguousarray(
                    adjv_s[:, :, s : s + B].reshape(128, KT * B)
                ).astype(bf16),
                "uft": np.ascontiguousarray(uftT[:, s : s + B]),
                "vft": np.ascontiguousarray(vftT[:, s : s + B]),
                "afac": chunked(a[s : s + B]),
                "bfac": chunked(b[s : s + B]),
            }
        )
    return in_maps


def assemble(results):
    U = np.empty((NU, OUT), np.float32)
    V = np.empty((NI, OUT), np.float32)
    for c in range(NCORES):
        U[c * B : (c + 1) * B] = results[c]["u_out"]
        V[c * B : (c + 1) * B] = results[c]["v_out"]
    return (U, V)


def kernel(**inputs):
    from concourse.bass_utils import run_bass_kernel_spmd

    nc = _get_program()
    res = run_bass_kernel_spmd(nc, make_in_maps(inputs), core_ids=list(range(NCORES)))
    return assemble(res.results)
